# revision 28
# baseline (speedup 1.0000x reference)
"""BinaryConv2d Trainium2 kernel (8-core batch-parallel, full 2x2 PE quadrant
packing).

Per image the 160 output rows split into 4 quarters of 40 rows; each quarter
is one 64x64 PE quadrant (tile_position (0,0)/(64,0)/(0,64)/(64,64)), so all
128x128 PEs are busy: quarters 0/2 stream from SBUF partitions 0:63/64:127 of
slabA, quarters 1/3 from slabB.  Each 3x3 position is 4 concurrent 64x64
matmuls accumulating into two PSUM tiles (bankA = quarters 0+1 on partitions
0:63/64:127, bankB = quarters 2+3).  Output is scaled into a per-image SBUF
accumulator (bf16) and stored with 4 large DMAs (~820 KB each); the final
fp32 upcast happens on host (bf16 rounding ~0.4%% << 2e-2 tolerance).

Input loads are SWDGE cast-DMAs (f32 HBM -> bf16 SBUF): under 8-core SPMD the
SBUF AXI ports (2:1 muxed between paired cores) are the contended resource at
~11-16 GB/s per SDMA engine, so halving the SBUF-side bytes halves the input
port cost.  Loads are software-pipelined one image ahead; image 0's
tile-0-critical rows go via HWDGE f32 with a Q7 gate holding back the SWDGE
bulk until they land (packet round-robin has no priority).
"""
import sys
import numpy as np
from contextlib import ExitStack

sys.path.insert(0, "/root/.axon_site/_ro/trn_rl_repo")
sys.path.insert(0, "/opt/trn_rl_repo")

import ml_dtypes
import concourse.bass as bass
import concourse.bacc as bacc
import concourse.mybir as mybir
import concourse.tile as tile
from concourse.bass_utils import run_bass_kernel_spmd

F32 = mybir.dt.float32
BF16 = mybir.dt.bfloat16

N_CORES = 8
B, CIN, COUT, KS = 32, 64, 64, 3
H = W = 160
B_CORE = B // N_CORES
QH = H // 4            # output rows per quarter (40)
SQ = QH + 2            # slab rows per quarter (1 halo/pad row each side)
PW = W + 2
RPT = 3                # output rows per PSUM tile


def build_nc(n_img=B_CORE, h=H, w=W):
    qh = h // 4
    sq = qh + 2
    pw = w + 2
    nc = bacc.Bacc("TRN2", target_bir_lowering=False, debug=False, num_devices=N_CORES)
    x_in = nc.declare_dram_parameter("x", [n_img, CIN, h, w], F32, isOutput=False)
    wsgn_in = nc.declare_dram_parameter("wsgn", [128, 9 * 64], BF16, isOutput=False)
    scale_in = nc.declare_dram_parameter("scale", [128, 1], F32, isOutput=False)
    out_ext = nc.declare_dram_parameter("out", [n_img, COUT, h, w], BF16, isOutput=True)

    n_tiles = (qh + RPT - 1) // RPT  # 13 full + 1 leftover row

    with tile.TileContext(nc) as tc, ExitStack() as ctx:
        wpool = ctx.enter_context(tc.tile_pool(name="wpool", bufs=1))
        spool = ctx.enter_context(tc.tile_pool(name="spool", bufs=1))
        xpool = ctx.enter_context(tc.tile_pool(name="xpool", bufs=3))
        ppool = ctx.enter_context(tc.tile_pool(name="ppool", bufs=3, space="PSUM"))
        opool = ctx.enter_context(tc.tile_pool(name="opool", bufs=2))

        wt2 = wpool.tile([128, 9 * 64], BF16, name="wt2")
        nc.sync.dma_start(wt2[:], wsgn_in[:])
        sc2 = wpool.tile([128, 1], F32, name="sc2")
        gdum = wpool.tile([128, 1], BF16, name="gdum")
        U16 = mybir.dt.uint16
        msk = wpool.tile([128, 1], U16, name="msk")
        nc.vector.memset(msk[:], 0x8000)
        one = wpool.tile([128, 1], U16, name="one")
        nc.vector.memset(one[:], 0x3F80)

        def sign_b(dst, src):
            # bitwise sign on bf16: (x & 0x8000) | 0x3F80 == +-1.0 exactly.
            # Runs on DVE to take the B-slab half of the binarize work off
            # ACT (the image-boundary critical path).
            bp, ps = dst.base_partition(), dst.partition_size()
            nc.vector.tensor_scalar(
                dst.bitcast(U16),
                src.bitcast(U16),
                msk[bp : bp + ps, :],
                one[bp : bp + ps, :],
                op0=mybir.AluOpType.bitwise_and,
                op1=mybir.AluOpType.bitwise_or,
            )

        # Two persistent slab pairs (manual ping-pong across images).
        # slabA: partitions 0:64 = quarter 0 (slab row s <- x row s-1),
        #        partitions 64:128 = quarter 2 (slab row s <- x row 2*qh-1+s)
        # slabB: partitions 0:64 = quarter 1 (slab row s <- x row qh-1+s),
        #        partitions 64:128 = quarter 3 (slab row s <- x row 3*qh-1+s)
        slabs = []
        for i in range(2):
            slabA = spool.tile([128, sq * pw], BF16, name=f"slabA{i}", tag=f"slabA{i}")
            slabB = spool.tile([128, sq * pw], BF16, name=f"slabB{i}", tag=f"slabB{i}")
            for slab in (slabA, slabB):
                # col pads: elements r*pw + {0, pw-1} for all slab rows
                nc.vector.memset(slab[:, 0 : (sq - 1) * pw + pw : pw], 0.0)
                nc.vector.memset(slab[:, pw - 1 : sq * pw : pw], 0.0)
            sA3 = slabA.rearrange("p (r c) -> p r c", c=pw)
            sB3 = slabB.rearrange("p (r c) -> p r c", c=pw)
            nc.vector.memset(sA3[0:64, 0, :], 0.0)        # image top pad
            nc.vector.memset(sB3[64:128, sq - 1, :], 0.0)  # image bottom pad
            slabs.append((slabA, slabB))

        def s3(img):
            slabA, slabB = slabs[img % 2]
            return (
                slabA.rearrange("p (r c) -> p r c", c=pw),
                slabB.rearrange("p (r c) -> p r c", c=pw),
            )

        # chunk split of the common slab-row ranges (A: rows 1..sq-1, B:
        # rows 0..sq-2).  Image 0 gets a small HWDGE f32 first chunk so tile
        # 0's matmuls start early; later images use coarse SWDGE chunks to
        # keep the emission count (and DMA queue-depth pressure) low.
        a_chunks0 = [(1, 6), (6, 24), (24, 42)]
        b_chunks0 = [(0, 5), (5, 23), (23, 41)]
        a_chunks = [(1, 22), (22, 42)]
        b_chunks = [(0, 21), (21, 41)]

        def emit_loads(img):
            # Returns deferred B-slab sign thunks (DVE).  They are injected
            # between the PREVIOUS image's tile groups: placing them directly
            # here would put them at the head of the DVE FIFO where, still
            # waiting on their prefetch DMA, they would block the current
            # image's PSUM drains (strict in-order queue).
            deferred = []
            first = img == 0
            sA3, sB3 = s3(img)
            dma_fst = nc.sync.dma_start if first else nc.gpsimd.dma_start
            ldt = F32 if first else BF16
            dtag = "f" if first else ""

            # top halo: slabA row 0 (bottom group) <- x row 2*qh-1; needed
            # by tile 0, so it leads.  (The bottom halo -> slabB row sq-1 is
            # only needed by tile 13 and is loaded after the bulk.)
            xs = xpool.tile([128, w], ldt, name="xs", tag=f"xs{dtag}")
            dma_fst(xs[64:128, :], x_in[img, :, 2 * qh - 1 : 2 * qh, :])
            nc.scalar.sign(sA3[64:128, 0, 1 : 1 + w], xs[64:128, :])

            ach = a_chunks0 if first else a_chunks
            bch = b_chunks0 if first else b_chunks
            for ci, ((a0, a1), (b0, b1)) in enumerate(zip(ach, bch)):
                crit = first and ci == 0
                ct = F32 if crit else BF16
                dma_in = nc.sync.dma_start if crit else nc.gpsimd.dma_start
                ctag = "f" if crit else ""
                na = a1 - a0
                xa = xpool.tile([128, na * w], ct, name="xa", tag=f"xc{na}{ctag}")
                xa3 = xa.rearrange("p (r c) -> p r c", c=w)
                dma_in(xa[0:64, :], x_in[img, :, a0 - 1 : a1 - 1, :])
                dma_in(
                    xa[64:128, :], x_in[img, :, 2 * qh - 1 + a0 : 2 * qh - 1 + a1, :]
                )
                nc.scalar.sign(sA3[:, a0:a1, 1 : 1 + w], xa3[:])

                nb = b1 - b0
                xb = xpool.tile([128, nb * w], ct, name="xb", tag=f"xc{nb}{ctag}")
                xb3 = xb.rearrange("p (r c) -> p r c", c=w)
                dma_in(xb[0:64, :], x_in[img, :, qh - 1 + b0 : qh - 1 + b1, :])
                dma_in(
                    xb[64:128, :], x_in[img, :, 3 * qh - 1 + b0 : 3 * qh - 1 + b1, :]
                )
                if crit:
                    nc.scalar.sign(sB3[:, b0:b1, 1 : 1 + w], xb3[:])
                else:
                    deferred.append(
                        lambda d=sB3[:, b0:b1, 1 : 1 + w], s=xb3[:]: sign_b(d, s)
                    )

                if crit:
                    # Q7 gate: stall SWDGE emission (and hence the bulk
                    # prefetch stream) until image 0's critical chunks have
                    # landed, so their packets get the SDMA engines to
                    # themselves and tile 0 starts early.
                    nc.gpsimd.tensor_scalar_add(gdum[0:64, :], xa[0:64, 0:1], 0.0)
                    nc.gpsimd.tensor_scalar_add(gdum[64:128, :], xb[64:128, 0:1], 0.0)
                    nc.sync.dma_start(sc2[:], scale_in[:])

            # bottom halo: slabB row sq-1 (top group) <- x row 2*qh
            xsb = xpool.tile([128, w], ldt, name="xsb", tag=f"xsb{dtag}")
            dma_fst(xsb[0:64, :], x_in[img, :, 2 * qh : 2 * qh + 1, :])
            if first:
                nc.scalar.sign(sB3[0:64, sq - 1, 1 : 1 + w], xsb[0:64, :])
            else:
                deferred.append(
                    lambda d=sB3[0:64, sq - 1, 1 : 1 + w], s=xsb[0:64, :]: sign_b(d, s)
                )
            return deferred

        def emit_compute_stores(img, pending=()):
            sA3, sB3 = s3(img)
            obufAB = opool.tile([128, qh * w], BF16, name="obufAB", tag="obufAB")
            obufCD = opool.tile([128, qh * w], BF16, name="obufCD", tag="obufCD")
            obs = [(obufAB, 0), (obufAB, 64), (obufCD, 0), (obufCD, 64)]
            last = img == n_img - 1

            for t in range(n_tiles):
                h0 = t * RPT
                R = min(RPT, qh - h0)
                psumA = ppool.tile([128, R * w], F32, name="psumA", tag="psumA")
                psumB = ppool.tile([128, R * w], F32, name="psumB", tag="psumB")
                for kh in range(KS):
                    for kw in range(KS):
                        pos = kh * KS + kw
                        st, sp = (pos == 0), (pos == 8)
                        wA = wt2[0:64, pos * 64 : (pos + 1) * 64]
                        wB = wt2[64:128, pos * 64 : (pos + 1) * 64]
                        nc.tensor.matmul(
                            psumA[0:64, :], wA,
                            sA3[0:64, h0 + kh : h0 + kh + R, kw : kw + w],
                            start=st, stop=sp, tile_position=(0, 0),
                        )
                        nc.tensor.matmul(
                            psumB[0:64, :], wB,
                            sA3[64:128, h0 + kh : h0 + kh + R, kw : kw + w],
                            start=st, stop=sp, tile_position=(64, 0),
                        )
                        nc.tensor.matmul(
                            psumA[64:128, :], wA,
                            sB3[0:64, h0 + kh : h0 + kh + R, kw : kw + w],
                            start=st, stop=sp, tile_position=(0, 64),
                        )
                        nc.tensor.matmul(
                            psumB[64:128, :], wB,
                            sB3[64:128, h0 + kh : h0 + kh + R, kw : kw + w],
                            start=st, stop=sp, tile_position=(64, 64),
                        )
                # scale + downcast into the per-image output accumulator (DVE)
                nc.vector.tensor_scalar_mul(
                    obufAB[:, h0 * w : (h0 + R) * w], psumA[:], sc2[:]
                )
                nc.vector.tensor_scalar_mul(
                    obufCD[:, h0 * w : (h0 + R) * w], psumB[:], sc2[:]
                )
                # next image's deferred B-signs, placed where their prefetch
                # data has already landed
                if pending and t in (5, 9, 11):
                    k = {5: 0, 9: 1, 11: 2}[t]
                    if k < len(pending):
                        pending[k]()
                # last image: flush finished row bands mid-compute so only
                # rows 36:40 remain as the post-compute DMA tail
                if last and t in (4, 8, 11):
                    lo = {4: 0, 8: 15, 11: 27}[t]
                    hi = h0 + R
                    for q, (ob, p0) in enumerate(obs):
                        nc.sync.dma_start(
                            out_ext[img, :, q * qh + lo : q * qh + hi, :],
                            ob[p0 : p0 + 64, lo * w : hi * w],
                        )

            if last:
                m = 36 * w  # rows 0:36 already stored
                for q, (ob, p0) in enumerate(obs):
                    nc.sync.dma_start(
                        out_ext[img, :, q * qh + 36 : (q + 1) * qh, :],
                        ob[p0 : p0 + 64, m:],
                    )
            else:
                for q, (ob, p0) in enumerate(obs):
                    nc.sync.dma_start(
                        out_ext[img, :, q * qh : (q + 1) * qh, :], ob[p0 : p0 + 64, :]
                    )

        for fn in emit_loads(0):
            fn()  # image 0's B-signs run inline (nothing queued before them)
        for img in range(n_img):
            pending = emit_loads(img + 1) if img + 1 < n_img else ()
            emit_compute_stores(img, pending)
    nc.finalize()
    return nc


_NC_CACHE = {}


def _get_nc():
    if "nc" not in _NC_CACHE:
        _NC_CACHE["nc"] = build_nc()
    return _NC_CACHE["nc"]


def _prep_weights(w):
    wc = np.clip(np.asarray(w, dtype=np.float32), -1.0, 1.0)
    scale = np.abs(wc).mean(axis=(1, 2, 3)).astype(np.float32).reshape(64, 1)
    s = np.sign(wc).astype(np.float32)  # [co, ci, kh, kw]
    wsgn = np.ascontiguousarray(
        np.transpose(s, (1, 2, 3, 0)).reshape(64, 9 * 64)
    )
    wsgn2 = np.concatenate([wsgn, wsgn], axis=0).astype(ml_dtypes.bfloat16)
    scale2 = np.concatenate([scale, scale], axis=0)
    return wsgn2, scale2


def kernel(x, w, _trace=False):
    x = np.ascontiguousarray(np.asarray(x, dtype=np.float32))
    wsgn2, scale2 = _prep_weights(w)
    nc = _get_nc()
    in_maps = [
        {"x": x[i * B_CORE : (i + 1) * B_CORE], "wsgn": wsgn2, "scale": scale2}
        for i in range(N_CORES)
    ]
    # The axon-proxied execution occasionally faults with a transient
    # NRT_EXEC_UNIT_UNRECOVERABLE; a retry on a fresh session recovers.
    last_err = None
    for attempt in range(3):
        try:
            res = run_bass_kernel_spmd(nc, in_maps, list(range(N_CORES)), trace=_trace)
            break
        except Exception as e:  # noqa: BLE001
            last_err = e
            import time as _time
            _time.sleep(3.0)
    else:
        raise last_err
    out = np.concatenate(
        [np.asarray(res.results[i]["out"]).astype(np.float32) for i in range(N_CORES)],
        axis=0,
    )
    if _trace:
        return out, res
    return out


# revision 29
# speedup vs baseline: 1.0979x; 1.0979x over previous
"""BinaryConv2d Trainium2 kernel (8-core batch-parallel, full 2x2 PE quadrant
packing).

Per image the 160 output rows split into 4 quarters of 40 rows; each quarter
is one 64x64 PE quadrant (tile_position (0,0)/(64,0)/(0,64)/(64,64)), so all
128x128 PEs are busy: quarters 0/2 stream from SBUF partitions 0:63/64:127 of
slabA, quarters 1/3 from slabB.  Each 3x3 position is 4 concurrent 64x64
matmuls accumulating into two PSUM tiles (bankA = quarters 0+1 on partitions
0:63/64:127, bankB = quarters 2+3).  Output is scaled into a per-image SBUF
accumulator (bf16) and stored with 4 large DMAs (~820 KB each); the final
fp32 upcast happens on host (bf16 rounding ~0.4%% << 2e-2 tolerance).

Input loads are SWDGE cast-DMAs (f32 HBM -> bf16 SBUF): under 8-core SPMD the
SBUF AXI ports (2:1 muxed between paired cores) are the contended resource at
~11-16 GB/s per SDMA engine, so halving the SBUF-side bytes halves the input
port cost.  Loads are software-pipelined one image ahead; image 0's
tile-0-critical rows go via HWDGE f32 with a Q7 gate holding back the SWDGE
bulk until they land (packet round-robin has no priority).
"""
import sys
import numpy as np
from contextlib import ExitStack

sys.path.insert(0, "/root/.axon_site/_ro/trn_rl_repo")
sys.path.insert(0, "/opt/trn_rl_repo")

import ml_dtypes
import concourse.bass as bass
import concourse.bacc as bacc
import concourse.mybir as mybir
import concourse.tile as tile
from concourse.bass_utils import run_bass_kernel_spmd

F32 = mybir.dt.float32
BF16 = mybir.dt.bfloat16

N_CORES = 8
B, CIN, COUT, KS = 32, 64, 64, 3
H = W = 160
B_CORE = B // N_CORES
QH = H // 4            # output rows per quarter (40)
SQ = QH + 2            # slab rows per quarter (1 halo/pad row each side)
PW = W + 2
RPT = 3                # output rows per PSUM tile


def build_nc(n_img=B_CORE, h=H, w=W):
    qh = h // 4
    sq = qh + 2
    pw = w + 2
    nc = bacc.Bacc("TRN2", target_bir_lowering=False, debug=False, num_devices=N_CORES)
    x_in = nc.declare_dram_parameter("x", [n_img, CIN, h, w], F32, isOutput=False)
    wsgn_in = nc.declare_dram_parameter("wsgn", [128, 9 * 64], BF16, isOutput=False)
    scale_in = nc.declare_dram_parameter("scale", [128, 1], F32, isOutput=False)
    out_ext = nc.declare_dram_parameter("out", [n_img, COUT, h, w], BF16, isOutput=True)

    n_tiles = (qh + RPT - 1) // RPT  # 13 full + 1 leftover row

    with tile.TileContext(nc) as tc, ExitStack() as ctx:
        wpool = ctx.enter_context(tc.tile_pool(name="wpool", bufs=1))
        spool = ctx.enter_context(tc.tile_pool(name="spool", bufs=1))
        xpool = ctx.enter_context(tc.tile_pool(name="xpool", bufs=3))
        ppool = ctx.enter_context(tc.tile_pool(name="ppool", bufs=3, space="PSUM"))
        opool = ctx.enter_context(tc.tile_pool(name="opool", bufs=2))

        wt2 = wpool.tile([128, 9 * 64], BF16, name="wt2")
        nc.sync.dma_start(wt2[:], wsgn_in[:])
        sc2 = wpool.tile([128, 1], F32, name="sc2")
        gdum = wpool.tile([128, 1], BF16, name="gdum")
        U16 = mybir.dt.uint16
        msk = wpool.tile([128, 1], U16, name="msk")
        nc.vector.memset(msk[:], 0x8000)
        one = wpool.tile([128, 1], U16, name="one")
        nc.vector.memset(one[:], 0x3F80)

        def sign_b(dst, src):
            # bitwise sign on bf16: (x & 0x8000) | 0x3F80 == +-1.0 exactly.
            # Runs on DVE to take the B-slab half of the binarize work off
            # ACT (the image-boundary critical path).
            bp, ps = dst.base_partition(), dst.partition_size()
            nc.vector.tensor_scalar(
                dst.bitcast(U16),
                src.bitcast(U16),
                msk[bp : bp + ps, :],
                one[bp : bp + ps, :],
                op0=mybir.AluOpType.bitwise_and,
                op1=mybir.AluOpType.bitwise_or,
            )

        # Two persistent slab pairs (manual ping-pong across images).
        # slabA: partitions 0:64 = quarter 0 (slab row s <- x row s-1),
        #        partitions 64:128 = quarter 2 (slab row s <- x row 2*qh-1+s)
        # slabB: partitions 0:64 = quarter 1 (slab row s <- x row qh-1+s),
        #        partitions 64:128 = quarter 3 (slab row s <- x row 3*qh-1+s)
        slabs = []
        for i in range(2):
            slabA = spool.tile([128, sq * pw], BF16, name=f"slabA{i}", tag=f"slabA{i}")
            slabB = spool.tile([128, sq * pw], BF16, name=f"slabB{i}", tag=f"slabB{i}")
            for slab in (slabA, slabB):
                # col pads: elements r*pw + {0, pw-1} for all slab rows
                nc.vector.memset(slab[:, 0 : (sq - 1) * pw + pw : pw], 0.0)
                nc.vector.memset(slab[:, pw - 1 : sq * pw : pw], 0.0)
            sA3 = slabA.rearrange("p (r c) -> p r c", c=pw)
            sB3 = slabB.rearrange("p (r c) -> p r c", c=pw)
            nc.vector.memset(sA3[0:64, 0, :], 0.0)        # image top pad
            nc.vector.memset(sB3[64:128, sq - 1, :], 0.0)  # image bottom pad
            slabs.append((slabA, slabB))

        def s3(img):
            slabA, slabB = slabs[img % 2]
            return (
                slabA.rearrange("p (r c) -> p r c", c=pw),
                slabB.rearrange("p (r c) -> p r c", c=pw),
            )

        # chunk split of the common slab-row ranges (A: rows 1..sq-1, B:
        # rows 0..sq-2).  Image 0 gets a small HWDGE f32 first chunk so tile
        # 0's matmuls start early; later images use coarse SWDGE chunks to
        # keep the emission count (and DMA queue-depth pressure) low.
        a_chunks0 = [(1, 6), (6, 24), (24, 42)]
        b_chunks0 = [(0, 5), (5, 23), (23, 41)]
        a_chunks = [(1, 9), (9, 26), (26, 42)]
        b_chunks = [(0, 8), (8, 25), (25, 41)]

        def emit_loads(img):
            # Returns deferred B-slab sign thunks (DVE).  They are injected
            # between the PREVIOUS image's tile groups: placing them directly
            # here would put them at the head of the DVE FIFO where, still
            # waiting on their prefetch DMA, they would block the current
            # image's PSUM drains (strict in-order queue).
            deferred = []
            first = img == 0
            sA3, sB3 = s3(img)
            dma_fst = nc.sync.dma_start if first else nc.gpsimd.dma_start
            ldt = F32 if first else BF16
            dtag = "f" if first else ""

            # top halo: slabA row 0 (bottom group) <- x row 2*qh-1; needed
            # by tile 0, so it leads.  (The bottom halo -> slabB row sq-1 is
            # only needed by tile 13 and is loaded after the bulk.)
            xs = xpool.tile([128, w], ldt, name="xs", tag=f"xs{dtag}")
            dma_fst(xs[64:128, :], x_in[img, :, 2 * qh - 1 : 2 * qh, :])
            nc.scalar.sign(sA3[64:128, 0, 1 : 1 + w], xs[64:128, :])

            ach = a_chunks0 if first else a_chunks
            bch = b_chunks0 if first else b_chunks
            for ci, ((a0, a1), (b0, b1)) in enumerate(zip(ach, bch)):
                crit = first and ci == 0
                ct = F32 if crit else BF16
                dma_in = nc.sync.dma_start if crit else nc.gpsimd.dma_start
                ctag = "f" if crit else ""
                na = a1 - a0
                xa = xpool.tile([128, na * w], ct, name="xa", tag=f"xc{na}{ctag}")
                xa3 = xa.rearrange("p (r c) -> p r c", c=w)
                dma_in(xa[0:64, :], x_in[img, :, a0 - 1 : a1 - 1, :])
                dma_in(
                    xa[64:128, :], x_in[img, :, 2 * qh - 1 + a0 : 2 * qh - 1 + a1, :]
                )
                nc.scalar.sign(sA3[:, a0:a1, 1 : 1 + w], xa3[:])

                nb = b1 - b0
                xb = xpool.tile([128, nb * w], ct, name="xb", tag=f"xc{nb}{ctag}")
                xb3 = xb.rearrange("p (r c) -> p r c", c=w)
                dma_in(xb[0:64, :], x_in[img, :, qh - 1 + b0 : qh - 1 + b1, :])
                dma_in(
                    xb[64:128, :], x_in[img, :, 3 * qh - 1 + b0 : 3 * qh - 1 + b1, :]
                )
                nc.scalar.sign(sB3[:, b0:b1, 1 : 1 + w], xb3[:])

                if crit:
                    # Q7 gate: stall SWDGE emission (and hence the bulk
                    # prefetch stream) until image 0's critical chunks have
                    # landed, so their packets get the SDMA engines to
                    # themselves and tile 0 starts early.
                    nc.gpsimd.tensor_scalar_add(gdum[0:64, :], xa[0:64, 0:1], 0.0)
                    nc.gpsimd.tensor_scalar_add(gdum[64:128, :], xb[64:128, 0:1], 0.0)
                    nc.sync.dma_start(sc2[:], scale_in[:])

            # bottom halo: slabB row sq-1 (top group) <- x row 2*qh
            xsb = xpool.tile([128, w], ldt, name="xsb", tag=f"xsb{dtag}")
            dma_fst(xsb[0:64, :], x_in[img, :, 2 * qh : 2 * qh + 1, :])
            nc.scalar.sign(sB3[0:64, sq - 1, 1 : 1 + w], xsb[0:64, :])
            return deferred

        def emit_compute_stores(img, pending=()):
            sA3, sB3 = s3(img)
            obufAB = opool.tile([128, qh * w], BF16, name="obufAB", tag="obufAB")
            obufCD = opool.tile([128, qh * w], BF16, name="obufCD", tag="obufCD")
            obs = [(obufAB, 0), (obufAB, 64), (obufCD, 0), (obufCD, 64)]
            last = img == n_img - 1

            for t in range(n_tiles):
                h0 = t * RPT
                R = min(RPT, qh - h0)
                psumA = ppool.tile([128, R * w], F32, name="psumA", tag="psumA")
                psumB = ppool.tile([128, R * w], F32, name="psumB", tag="psumB")
                for kh in range(KS):
                    for kw in range(KS):
                        pos = kh * KS + kw
                        st, sp = (pos == 0), (pos == 8)
                        wA = wt2[0:64, pos * 64 : (pos + 1) * 64]
                        wB = wt2[64:128, pos * 64 : (pos + 1) * 64]
                        nc.tensor.matmul(
                            psumA[0:64, :], wA,
                            sA3[0:64, h0 + kh : h0 + kh + R, kw : kw + w],
                            start=st, stop=sp, tile_position=(0, 0),
                        )
                        nc.tensor.matmul(
                            psumB[0:64, :], wB,
                            sA3[64:128, h0 + kh : h0 + kh + R, kw : kw + w],
                            start=st, stop=sp, tile_position=(64, 0),
                        )
                        nc.tensor.matmul(
                            psumA[64:128, :], wA,
                            sB3[0:64, h0 + kh : h0 + kh + R, kw : kw + w],
                            start=st, stop=sp, tile_position=(0, 64),
                        )
                        nc.tensor.matmul(
                            psumB[64:128, :], wB,
                            sB3[64:128, h0 + kh : h0 + kh + R, kw : kw + w],
                            start=st, stop=sp, tile_position=(64, 64),
                        )
                # scale + downcast into the per-image output accumulator (DVE)
                nc.vector.tensor_scalar_mul(
                    obufAB[:, h0 * w : (h0 + R) * w], psumA[:], sc2[:]
                )
                nc.vector.tensor_scalar_mul(
                    obufCD[:, h0 * w : (h0 + R) * w], psumB[:], sc2[:]
                )
                # last image: flush finished row bands mid-compute so only
                # rows 36:40 remain as the post-compute DMA tail
                if last and t in (4, 8, 11):
                    lo = {4: 0, 8: 15, 11: 27}[t]
                    hi = h0 + R
                    for q, (ob, p0) in enumerate(obs):
                        nc.sync.dma_start(
                            out_ext[img, :, q * qh + lo : q * qh + hi, :],
                            ob[p0 : p0 + 64, lo * w : hi * w],
                        )

            if last:
                m = 36 * w  # rows 0:36 already stored
                for q, (ob, p0) in enumerate(obs):
                    nc.sync.dma_start(
                        out_ext[img, :, q * qh + 36 : (q + 1) * qh, :],
                        ob[p0 : p0 + 64, m:],
                    )
            else:
                for q, (ob, p0) in enumerate(obs):
                    nc.sync.dma_start(
                        out_ext[img, :, q * qh : (q + 1) * qh, :], ob[p0 : p0 + 64, :]
                    )

        emit_loads(0)
        for img in range(n_img):
            if img + 1 < n_img:
                emit_loads(img + 1)
            emit_compute_stores(img)
    nc.finalize()
    return nc


_NC_CACHE = {}


def _get_nc():
    if "nc" not in _NC_CACHE:
        _NC_CACHE["nc"] = build_nc()
    return _NC_CACHE["nc"]


def _prep_weights(w):
    wc = np.clip(np.asarray(w, dtype=np.float32), -1.0, 1.0)
    scale = np.abs(wc).mean(axis=(1, 2, 3)).astype(np.float32).reshape(64, 1)
    s = np.sign(wc).astype(np.float32)  # [co, ci, kh, kw]
    wsgn = np.ascontiguousarray(
        np.transpose(s, (1, 2, 3, 0)).reshape(64, 9 * 64)
    )
    wsgn2 = np.concatenate([wsgn, wsgn], axis=0).astype(ml_dtypes.bfloat16)
    scale2 = np.concatenate([scale, scale], axis=0)
    return wsgn2, scale2


def kernel(x, w, _trace=False):
    x = np.ascontiguousarray(np.asarray(x, dtype=np.float32))
    wsgn2, scale2 = _prep_weights(w)
    nc = _get_nc()
    in_maps = [
        {"x": x[i * B_CORE : (i + 1) * B_CORE], "wsgn": wsgn2, "scale": scale2}
        for i in range(N_CORES)
    ]
    # The axon-proxied execution occasionally faults with a transient
    # NRT_EXEC_UNIT_UNRECOVERABLE; a retry on a fresh session recovers.
    last_err = None
    for attempt in range(3):
        try:
            res = run_bass_kernel_spmd(nc, in_maps, list(range(N_CORES)), trace=_trace)
            break
        except Exception as e:  # noqa: BLE001
            last_err = e
            import time as _time
            _time.sleep(3.0)
    else:
        raise last_err
    out = np.concatenate(
        [np.asarray(res.results[i]["out"]).astype(np.float32) for i in range(N_CORES)],
        axis=0,
    )
    if _trace:
        return out, res
    return out
